# revision 28
# baseline (speedup 1.0000x reference)
"""KNN classifier layer (B=1024, N=32768, D=64, k=8, C=6) on 8 trn2 cores.

Strategy: shard queries (batch) across the 8 cores, 128 queries per core;
replicate the training set. Per core:
  key[q, n] = x_q . X_n - |X_n|^2/2   (monotone decreasing in distance^2)
computed as one augmented matmul ([x, 1] . [X, -|X|^2/2]) in float32r
(full-rate PE). X_train is pre-sorted by class on the host and each class
block is padded to a multiple of 512 columns, so every 512-column matmul
chunk belongs to exactly one class. The DVE top-8 (max8) runs directly on
PSUM per (macro-tile x class) region; the per-region top-8 candidates carry
their class statically. Global top-8 = max8 over all candidates; per-class
neighbor counts = #(class candidates >= t_q) where t_q is the 8th largest
key, computed with one tiny fused is_ge+accumulate per class. No PSUM
evacuation, no full-array rescan. Counts are written transposed ([C, Q])
so the output DMA is 6 fat descriptors; normalization happens on host.
"""

import numpy as np

B, N, D, K, C = 1024, 32768, 64, 8, 6
NCORES = 8
Q = B // NCORES  # queries per core

CHUNK = 512    # matmul moving free dim (one PSUM bank)
MACRO = 2048   # PSUM tile width (4 banks), 2 tiles in flight
DMA_W = 1536   # xm DMA chunk width (columns, multiple of CHUNK)
NEG = -1.0e30

_compiled = None
_compiled_key = None


def _plan_layout(y_train: np.ndarray):
    """Class-sort permutation; class blocks padded to multiples of 8 columns
    (max8 regions have no alignment constraint beyond sanity, matmul chunks
    ignore class boundaries); total padded to a CHUNK multiple with dead
    columns that belong to no region."""
    perm = np.argsort(y_train, kind="stable")
    counts = np.bincount(y_train, minlength=C)
    widths = [int(-(-c // 8) * 8) for c in counts]
    starts = np.concatenate([[0], np.cumsum(widths)]).astype(int)
    np_cols = int(-(-int(starts[-1]) // CHUNK) * CHUNK)
    return perm, counts, widths, starts, np_cols


def _macro_schedule(np_cols):
    """PSUM macro tile widths: small ramp first so the DVE starts early,
    then full 2048-column (4-bank) tiles."""
    ws = []
    for w in (512, 512, 1024):
        if sum(ws) + w <= np_cols:
            ws.append(w)
    while np_cols - sum(ws) >= MACRO:
        ws.append(MACRO)
    if np_cols - sum(ws):
        ws.append(np_cols - sum(ws))
    starts = np.concatenate([[0], np.cumsum(ws)]).astype(int)
    return [(int(starts[i]), int(ws[i])) for i in range(len(ws))]


def _chunk_is_3mm(ci):
    """Per-512-chunk matmul scheme: every 4th chunk uses the 3-matmul
    shared-tile form (light DMA, heavy PE), the rest the 2-matmul
    packed-dup form (light PE, heavy DMA). The 3:1 mix keeps each PSUM
    macro's PE time ~300ns under the DVE max8 time (absorbing semaphore
    latency in the 2-deep PSUM pipeline) while holding DMA ~15% under the
    DVE scan rate."""
    return ci % 4 == 3


def _dma_groups(n_chunks):
    """Chunk-count per DMA group: small ramp so the first matmul starts
    early, then 8-chunk groups."""
    gs = []
    for g in (2, 2, 4):
        if sum(gs) + g <= n_chunks:
            gs.append(g)
    while n_chunks - sum(gs) >= 8:
        gs.append(8)
    if n_chunks - sum(gs):
        gs.append(n_chunks - sum(gs))
    bounds = np.concatenate([[0], np.cumsum(gs)]).astype(int)
    return [(int(bounds[i]), int(gs[i])) for i in range(len(gs))]


def _regions(starts, macros):
    """Per (macro, class) regions: list of (macro_idx, class, start_in_macro,
    width)."""
    regs = []
    for c in range(C):
        lo, hi = int(starts[c]), int(starts[c + 1])
        for mi, (ms, mw) in enumerate(macros):
            s = max(lo, ms)
            e = min(hi, ms + mw)
            if s < e:
                regs.append((mi, c, s - ms, e - s))
    return regs


def _build_nc(np_cols: int, starts):
    """fp16 compensated matmul: key = xh.th + xl.th + xh.tl computed as two
    PSUM-accumulated matmuls per 512-chunk via contraction packing:
      mmA: [xh(65); xl(63)] . [th(65); th[0:63]]   (128 rows)
      mmB: [xh(65); xl[63]] . [tl(65); th[63]]     (66 rows)
    (xl's bias-row residual is exactly 0 so only 64 xl feature rows matter.)
    Max abs key error ~2e-5 (probed on HW), vs fp32's 4-cycle-per-row cost.
    """
    import concourse.bacc as bacc
    import concourse.mybir as mybir
    from concourse.tile import TileContext

    f32 = mybir.dt.float32
    f16 = mybir.dt.float16
    nc = bacc.Bacc(None, target_bir_lowering=False, debug=False)

    n_chunks = np_cols // CHUNK
    colsA = sum(CHUNK for ci in range(n_chunks) if not _chunk_is_3mm(ci))
    colsH = np_cols - colsA

    lA_d = nc.declare_dram_parameter("lA", [128, Q], f16, isOutput=False)
    lB_d = nc.declare_dram_parameter("lB", [D + 2, Q], f16, isOutput=False)
    lL_d = nc.declare_dram_parameter("lL", [D + 1, Q], f16, isOutput=False)
    xmA_d = nc.declare_dram_parameter("xmA", [128, colsA], f16, isOutput=False)
    xmB_d = nc.declare_dram_parameter("xmB", [D + 2, colsA], f16, isOutput=False)
    xmH_d = nc.declare_dram_parameter("xmH", [D + 1, colsH], f16, isOutput=False)
    xmL_d = nc.declare_dram_parameter("xmL", [D + 1, colsH], f16, isOutput=False)
    out_d = nc.declare_dram_parameter("out", [C, Q], f32, isOutput=True)

    macros = _macro_schedule(np_cols)
    regs = _regions(starts, macros)
    # uniform class-major candidate slots: class c owns slots
    # [c*spc, (c+1)*spc); unused slots stay at NEG (memset) and never count
    spc = max(sum(1 for r in regs if r[1] == c) for c in range(C))
    n_slots = C * spc
    slot_of_reg = {}
    per_class_seen = [0] * C
    for r in regs:
        _, c, _, _ = r
        slot_of_reg[r] = c * spc + per_class_seen[c]
        per_class_seen[c] += 1

    with TileContext(nc) as tc:
        with (
            tc.tile_pool(name="const", bufs=1) as const_pool,
            tc.tile_pool(name="rhs", bufs=4) as rhs_pool,
            tc.tile_pool(name="psum", bufs=2, space="PSUM") as psum_pool,
            tc.tile_pool(name="small", bufs=1) as small_pool,
        ):
            lA_sb = const_pool.tile([128, Q], f16)
            nc.sync.dma_start(out=lA_sb, in_=lA_d[:, :])
            lB_sb = const_pool.tile([D + 2, Q], f16)
            nc.sync.dma_start(out=lB_sb, in_=lB_d[:, :])
            lL_sb = const_pool.tile([D + 1, Q], f16)
            nc.sync.dma_start(out=lL_sb, in_=lL_d[:, :])

            cand = small_pool.tile([Q, n_slots * 8], f32)
            nc.vector.memset(cand[:, :], NEG)
            cnt32 = small_pool.tile([Q, 32], f32)
            nc.vector.memset(cnt32[:, :], 0.0)

            # DMA groups of consecutive chunks: 2mm chunks go to an A/B
            # tile pair, 3mm chunks to an H/L pair; scheme tensors are
            # column-compacted on the host in the same order.
            groups = _dma_groups(n_chunks)
            tileA, tileH, group_of = [], [], {}
            for gi, (c0, gn) in enumerate(groups):
                cis = list(range(c0, c0 + gn))
                for ci in cis:
                    group_of[ci] = gi
                wA = sum(CHUNK for ci in cis if not _chunk_is_3mm(ci))
                wH = sum(CHUNK for ci in cis if _chunk_is_3mm(ci))
                offA = sum(CHUNK for ci in range(c0) if not _chunk_is_3mm(ci))
                offH = sum(CHUNK for ci in range(c0) if _chunk_is_3mm(ci))
                pairA = pairH = None
                if wA:
                    rtA = rhs_pool.tile([128, wA], f16, tag="rtA")
                    nc.sync.dma_start(out=rtA, in_=xmA_d[:, offA : offA + wA])
                    rtB = rhs_pool.tile([D + 2, wA], f16, tag="rtB")
                    nc.sync.dma_start(out=rtB, in_=xmB_d[:, offA : offA + wA])
                    pairA = (rtA, rtB)
                if wH:
                    rtH = rhs_pool.tile([D + 1, wH], f16, tag="rtH")
                    nc.sync.dma_start(out=rtH, in_=xmH_d[:, offH : offH + wH])
                    rtL = rhs_pool.tile([D + 1, wH], f16, tag="rtL")
                    nc.sync.dma_start(out=rtL, in_=xmL_d[:, offH : offH + wH])
                    pairH = (rtH, rtL)
                tileA.append(pairA)
                tileH.append(pairH)

            for mi, (ms, mw) in enumerate(macros):
                ps = psum_pool.tile([Q, mw], f32, tag="ps")
                for j in range(mw // CHUNK):
                    ci = (ms + j * CHUNK) // CHUNK
                    g = group_of[ci]
                    c0 = groups[g][0]
                    sl = slice(j * CHUNK, (j + 1) * CHUNK)
                    if not _chunk_is_3mm(ci):
                        rtA, rtB = tileA[g]
                        off = sum(CHUNK for k in range(c0, ci)
                                  if not _chunk_is_3mm(k))
                        rsl = slice(off, off + CHUNK)
                        nc.tensor.matmul(
                            ps[:, sl], lhsT=lA_sb, rhs=rtA[:, rsl],
                            start=True, stop=False,
                        )
                        nc.tensor.matmul(
                            ps[:, sl], lhsT=lB_sb, rhs=rtB[:, rsl],
                            start=False, stop=True,
                        )
                    else:
                        rtH, rtL = tileH[g]
                        off = sum(CHUNK for k in range(c0, ci)
                                  if _chunk_is_3mm(k))
                        rsl = slice(off, off + CHUNK)
                        nc.tensor.matmul(
                            ps[:, sl], lhsT=lA_sb[0 : D + 1, :], rhs=rtH[:, rsl],
                            start=True, stop=False,
                        )
                        nc.tensor.matmul(
                            ps[:, sl], lhsT=lL_sb, rhs=rtH[:, rsl],
                            start=False, stop=False,
                        )
                        nc.tensor.matmul(
                            ps[:, sl], lhsT=lA_sb[0 : D + 1, :], rhs=rtL[:, rsl],
                            start=False, stop=True,
                        )
                # per-class top8 regions of this macro, straight from PSUM
                for r in regs:
                    rm, rc, rs, rw = r
                    if rm != mi:
                        continue
                    s = slot_of_reg[r]
                    nc.vector.max(
                        out=cand[:, s * 8 : (s + 1) * 8], in_=ps[:, rs : rs + rw]
                    )

            v8 = small_pool.tile([Q, 8], f32)
            nc.vector.max(out=v8, in_=cand)
            tq = v8[:, 7:8]

            # one fused is_ge over all candidates, then one windowed reduce
            # (sum per class block of spc*8 flags) -> per-class counts
            scr = small_pool.tile([Q, n_slots * 8], f32)
            nc.vector.tensor_scalar(
                out=scr[:, :],
                in0=cand[:, :],
                scalar1=tq,
                scalar2=None,
                op0=mybir.AluOpType.is_ge,
            )
            nc.vector.tensor_reduce(
                out=cnt32[:, 0:C],
                in_=scr[:, :].rearrange("p (c w) -> p c w", w=spc * 8),
                op=mybir.AluOpType.add,
                axis=mybir.AxisListType.X,
            )

            # transpose [128, 32] -> [32, 128] via 4 stream-transposed blocks
            cntT = small_pool.tile([32, Q], f32)
            for jb in range(4):
                nc.vector.transpose(
                    out=cntT[:, jb * 32 : (jb + 1) * 32],
                    in_=cnt32[jb * 32 : (jb + 1) * 32, :],
                )
            nc.sync.dma_start(out=out_d[:, :], in_=cntT[0:C, :])

    nc.finalize()
    return nc


def _prepare(x: np.ndarray, X_train: np.ndarray, y_train: np.ndarray):
    perm, counts, widths, starts, np_cols = _plan_layout(y_train)
    Xs = X_train[perm].astype(np.float32)  # [N, D] class-sorted
    t_sq = np.sum(Xs * Xs, axis=1)

    that = np.zeros((D + 1, np_cols), dtype=np.float32)
    # fp16-representable pad bias; real keys are O(100), so pad columns
    # never reach any top-8 (NEG would overflow the fp16 split to inf/NaN)
    that[D, :] = -60000.0
    pos = 0
    for ci in range(C):
        s = int(starts[ci])
        cnt_c = int(counts[ci])
        sel = slice(pos, pos + cnt_c)
        that[:D, s : s + cnt_c] = Xs[sel].T
        that[D, s : s + cnt_c] = -0.5 * t_sq[sel]
        pos += cnt_c

    th = that.astype(np.float16)
    tl = (that - th.astype(np.float32)).astype(np.float16)

    n_chunks = np_cols // CHUNK
    selA = np.concatenate(
        [np.arange(ci * CHUNK, (ci + 1) * CHUNK) for ci in range(n_chunks)
         if not _chunk_is_3mm(ci)])
    selH = np.concatenate(
        [np.arange(ci * CHUNK, (ci + 1) * CHUNK) for ci in range(n_chunks)
         if _chunk_is_3mm(ci)])
    xmA = np.concatenate([th[:, selA], th[0:63, selA]], axis=0)  # [128, colsA]
    xmB = np.concatenate([tl[:, selA], th[63:64, selA]], axis=0)  # [66, colsA]
    xmH = th[:, selH]                                             # [65, colsH]
    xmL = tl[:, selH]                                             # [65, colsH]
    return xmA, xmB, xmH, xmL, starts, np_cols


def _prep_queries(x: np.ndarray, core: int):
    xc = x[core * Q : (core + 1) * Q].astype(np.float32)  # [Q, D]
    xhat = np.concatenate([xc.T, np.ones((1, Q), np.float32)], axis=0)
    xh = xhat.astype(np.float16)
    xl = (xhat - xh.astype(np.float32)).astype(np.float16)
    lA = np.concatenate([xh, xl[0:63]], axis=0)           # [128, Q]
    lB = np.concatenate([xh, xl[63:64]], axis=0)          # [66, Q]
    lL = xl                                               # [65, Q]
    return lA, lB, lL


def kernel(x: np.ndarray, X_train: np.ndarray, y_train: np.ndarray) -> np.ndarray:
    global _compiled, _compiled_key
    from concourse.bass_utils import run_bass_kernel_spmd

    xmA, xmB, xmH, xmL, starts, np_cols = _prepare(x, X_train, y_train)

    key = (np_cols, tuple(int(s) for s in starts))
    if _compiled is None or _compiled_key != key:
        _compiled = _build_nc(np_cols, starts)
        _compiled_key = key
    nc = _compiled

    in_maps = []
    for core in range(NCORES):
        lA, lB, lL = _prep_queries(x, core)
        in_maps.append({"lA": lA, "lB": lB, "lL": lL,
                        "xmA": xmA, "xmB": xmB, "xmH": xmH, "xmL": xmL})

    res = run_bass_kernel_spmd(nc, in_maps, core_ids=list(range(NCORES)))
    counts = np.concatenate(
        [res.results[i]["out"].T for i in range(NCORES)], axis=0
    )  # [B, C] neighbor counts
    out = counts / np.sum(counts, axis=-1, keepdims=True)
    return out.astype(np.float32)


# revision 49
# speedup vs baseline: 1.1076x; 1.1076x over previous
"""KNN classifier layer (B=1024, N=32768, D=64, k=8, C=6) on 8 trn2 cores.

Sharding: X_train is split 8 ways (per class, evenly); every core scores
ALL 1024 queries (8 partition-blocks of 128) against its ~4096-point
shard. key[q,n] = x_q.X_n - |X_n|^2/2 via an fp16-compensated augmented
matmul (xh.th + xl.th + xh.tl, packed into 2 PSUM-accumulated matmuls by
contraction packing; max |key| error ~2e-5, probed on HW). The per-core
layout is class-sorted with identical per-class widths on every core
(SPMD-uniform). Per (query-block, class-pair) PSUM tile, the DVE max8
runs directly on PSUM per class region; the 6 per-class top-8 candidate
vectors per query-block land in one SBUF array that is DMA'd out once.
The host merges the 8 cores' candidates: global top-8 threshold t_q per
query, per-class counts = #(class candidates >= t_q), normalize. Device
does the O(B.N) work; host merge is O(B x 384).
"""

import numpy as np

B, N, D, K, C = 1024, 32768, 64, 8, 6
NCORES = 8
Q = 128          # queries per partition block
NQB = B // Q     # query blocks per core (all cores see all queries)
CHUNK = 512      # max matmul moving free dim
NEG_PAD = -60000.0   # fp16-representable pad bias; real keys are O(100)

_compiled = None
_compiled_key = None


def _plan_layout(y_train: np.ndarray):
    """Per-core class widths (uniform across cores, multiples of 8)."""
    perm = np.argsort(y_train, kind="stable")
    counts = np.bincount(y_train, minlength=C)
    q_c = [int(-(-int(c) // NCORES)) for c in counts]        # per-core members
    W = [int(-(-qc // 8) * 8) for qc in q_c]                 # padded widths
    starts = np.concatenate([[0], np.cumsum(W)]).astype(int)
    pcp = int(starts[-1])                                    # per-core columns
    return perm, counts, q_c, W, starts, pcp, pcp


def _class_tiles(W):
    """PSUM tiles each covering a pair of classes: [(class_list, width)]."""
    tiles = []
    for c in range(0, C, 2):
        tiles.append(((c, c + 1), W[c] + W[c + 1]))
    return tiles


def _build_nc(W, pcp):
    import concourse.bacc as bacc
    import concourse.mybir as mybir
    from concourse.tile import TileContext

    f32 = mybir.dt.float32
    f16 = mybir.dt.float16
    nc = bacc.Bacc(None, target_bir_lowering=False, debug=False)

    lA_d = nc.declare_dram_parameter("lA", [128, B], f16, isOutput=False)
    lB_d = nc.declare_dram_parameter("lB", [D + 2, B], f16, isOutput=False)
    xmA_d = nc.declare_dram_parameter("xmA", [128, pcp], f16, isOutput=False)
    xmB_d = nc.declare_dram_parameter("xmB", [D + 2, pcp], f16, isOutput=False)
    out_d = nc.declare_dram_parameter("out", [Q, NQB * C * 8], f32, isOutput=True)

    tiles = _class_tiles(W)
    starts = np.concatenate([[0], np.cumsum(W)]).astype(int)

    with TileContext(nc) as tc:
        with (
            tc.tile_pool(name="const", bufs=1) as const_pool,
            tc.tile_pool(name="psum", bufs=2, space="PSUM") as psum_pool,
            tc.tile_pool(name="small", bufs=3) as small_pool,
        ):
            lA_sb = const_pool.tile([128, B], f16)
            lB_sb = const_pool.tile([D + 2, B], f16)
            xmA_sb = const_pool.tile([128, pcp], f16)
            xmB_sb = const_pool.tile([D + 2, pcp], f16)
            # DMA order: exactly what the first (qblock 0, tile 0) matmuls
            # need, then everything else; xm split on tile boundaries
            h1 = int(starts[2])
            nc.sync.dma_start(out=lA_sb[:, 0:Q], in_=lA_d[:, 0:Q])
            nc.sync.dma_start(out=lB_sb[:, 0:Q], in_=lB_d[:, 0:Q])
            nc.sync.dma_start(out=xmA_sb[:, 0:h1], in_=xmA_d[:, 0:h1])
            nc.sync.dma_start(out=xmB_sb[:, 0:h1], in_=xmB_d[:, 0:h1])
            h2 = int(starts[4])
            nc.sync.dma_start(out=xmA_sb[:, h1:h2], in_=xmA_d[:, h1:h2])
            nc.sync.dma_start(out=xmB_sb[:, h1:h2], in_=xmB_d[:, h1:h2])
            nc.sync.dma_start(out=lA_sb[:, Q:B], in_=lA_d[:, Q:B])
            nc.sync.dma_start(out=lB_sb[:, Q:B], in_=lB_d[:, Q:B])
            nc.sync.dma_start(out=xmA_sb[:, h2:pcp], in_=xmA_d[:, h2:pcp])
            nc.sync.dma_start(out=xmB_sb[:, h2:pcp], in_=xmB_d[:, h2:pcp])

            for b in range(NQB):
                lAb = lA_sb[:, b * Q : (b + 1) * Q]
                lBb = lB_sb[:, b * Q : (b + 1) * Q]
                candb = small_pool.tile([Q, C * 8], f32, tag="cand")
                for (cls, tw) in tiles:
                    t0 = int(starts[cls[0]])
                    ps = psum_pool.tile([Q, tw], f32, tag="ps")
                    pos = 0
                    while pos < tw:
                        w = min(CHUNK, tw - pos)
                        sl = slice(pos, pos + w)
                        rsl = slice(t0 + pos, t0 + pos + w)
                        nc.tensor.matmul(
                            ps[:, sl], lhsT=lAb, rhs=xmA_sb[:, rsl],
                            start=True, stop=False,
                        )
                        nc.tensor.matmul(
                            ps[:, sl], lhsT=lBb, rhs=xmB_sb[:, rsl],
                            start=False, stop=True,
                        )
                        pos += w
                    for c in cls:
                        rs = int(starts[c]) - t0
                        nc.vector.max(
                            out=candb[:, c * 8 : (c + 1) * 8],
                            in_=ps[:, rs : rs + W[c]],
                        )
                bs = b * C * 8
                nc.sync.dma_start(out=out_d[:, bs : bs + C * 8], in_=candb)

    nc.finalize()
    return nc


def _prepare(x, X_train, y_train):
    perm, counts, q_c, W, starts, pc, pcp = _plan_layout(y_train)
    Xs = X_train[perm].astype(np.float32)
    t_sq = np.sum(Xs * Xs, axis=1)

    # per-core augmented columns [core][65, pcp]
    that = np.zeros((NCORES, D + 1, pcp), dtype=np.float32)
    that[:, D, :] = NEG_PAD
    cstart = np.concatenate([[0], np.cumsum(counts)]).astype(int)
    for c in range(C):
        for k in range(NCORES):
            lo = int(cstart[c]) + k * q_c[c]
            hi = min(int(cstart[c]) + (k + 1) * q_c[c], int(cstart[c + 1]))
            n = hi - lo
            if n <= 0:
                continue
            s = int(starts[c])
            that[k, :D, s : s + n] = Xs[lo:hi].T
            that[k, D, s : s + n] = -0.5 * t_sq[lo:hi]

    th = that.astype(np.float16)
    tl = (that - th.astype(np.float32)).astype(np.float16)
    xmA = np.concatenate([th, th[:, 0:63]], axis=1)      # [8, 128, pcp]
    xmB = np.concatenate([tl, th[:, 63:64]], axis=1)     # [8, 66, pcp]

    xhat = np.concatenate([x.T.astype(np.float32), np.ones((1, B), np.float32)], axis=0)
    xh = xhat.astype(np.float16)
    xl = (xhat - xh.astype(np.float32)).astype(np.float16)
    lA = np.concatenate([xh, xl[0:63]], axis=0)          # [128, B]
    lB = np.concatenate([xh, xl[63:64]], axis=0)         # [66, B]
    return xmA, xmB, lA, lB, W, pcp


def kernel(x: np.ndarray, X_train: np.ndarray, y_train: np.ndarray) -> np.ndarray:
    global _compiled, _compiled_key
    from concourse.bass_utils import run_bass_kernel_spmd

    xmA, xmB, lA, lB, W, pcp = _prepare(x, X_train, y_train)

    key = (pcp, tuple(W))
    if _compiled is None or _compiled_key != key:
        _compiled = _build_nc(W, pcp)
        _compiled_key = key
    nc = _compiled

    in_maps = [
        {"lA": lA, "lB": lB, "xmA": xmA[k], "xmB": xmB[k]}
        for k in range(NCORES)
    ]
    res = run_bass_kernel_spmd(nc, in_maps, core_ids=list(range(NCORES)))

    # host merge: [cores][Q, NQB*C*8] -> per query 8*C*8 candidates
    outs = np.stack([res.results[k]["out"] for k in range(NCORES)])  # [8,Q,NQB*C*8]
    outs = outs.reshape(NCORES, Q, NQB, C * 8).transpose(2, 1, 0, 3)  # [NQB,Q,8,C*8]
    vals = outs.reshape(B, NCORES * C * 8)
    # class of each slot: slots are [core][class][8]
    cls = np.tile(np.repeat(np.arange(C), 8), NCORES)                 # [NCORES*C*8]
    tq = -np.partition(-vals, K - 1, axis=1)[:, K - 1 : K]            # [B,1]
    ge = vals >= tq                                                   # [B, S]
    counts = np.stack([np.sum(ge[:, cls == c], axis=1) for c in range(C)], axis=1)
    counts = counts.astype(np.float32)
    out = counts / np.sum(counts, axis=-1, keepdims=True)
    return out.astype(np.float32)


# revision 57
# speedup vs baseline: 1.2226x; 1.1039x over previous
"""KNN classifier layer (B=1024, N=32768, D=64, k=8, C=6) on 8 trn2 cores.

Sharding: X_train is split 8 ways (per class, evenly); every core scores
ALL 1024 queries (8 partition-blocks of 128) against its ~4096-point
shard. key[q,n] = x_q.X_n - |X_n|^2/2 via an fp16-compensated augmented
matmul (xh.th + xl.th + xh.tl, packed into 2 PSUM-accumulated matmuls by
contraction packing; max |key| error ~2e-5, probed on HW). The per-core
layout is class-sorted with identical per-class widths on every core
(SPMD-uniform). Per (query-block, class-pair) PSUM tile, the DVE max8
runs directly on PSUM per class region; the 6 per-class top-8 candidate
vectors per query-block land in one SBUF array that is DMA'd out once.
The host merges the 8 cores' candidates: global top-8 threshold t_q per
query, per-class counts = #(class candidates >= t_q), normalize. Device
does the O(B.N) work; host merge is O(B x 384).
"""

import numpy as np

B, N, D, K, C = 1024, 32768, 64, 8, 6
NCORES = 8
Q = 128          # queries per partition block
NQB = B // Q     # query blocks per core (all cores see all queries)
CHUNK = 512      # max matmul moving free dim
NEG_PAD = -60000.0   # fp16-representable pad bias; real keys are O(100)

_compiled = None
_compiled_key = None


def _plan_layout(y_train: np.ndarray):
    """Per-core class widths (uniform across cores, multiples of 8)."""
    perm = np.argsort(y_train, kind="stable")
    counts = np.bincount(y_train, minlength=C)
    q_c = [int(-(-int(c) // NCORES)) for c in counts]        # per-core members
    W = [max(8, int(-(-qc // 8) * 8)) for qc in q_c]         # padded widths
    starts = np.concatenate([[0], np.cumsum(W)]).astype(int)
    pcp = int(starts[-1])                                    # per-core columns
    return perm, counts, q_c, W, starts, pcp, pcp


def _class_tiles(W):
    """PSUM tiles, one class each (2 banks -> 4-deep PSUM pipeline)."""
    return [((c,), W[c]) for c in range(C)]


def _build_nc(W, pcp):
    import concourse.bacc as bacc
    import concourse.mybir as mybir
    from concourse.tile import TileContext

    f32 = mybir.dt.float32
    f16 = mybir.dt.float16
    nc = bacc.Bacc(None, target_bir_lowering=False, debug=False)

    lA_d = nc.declare_dram_parameter("lA", [128, B], f16, isOutput=False)
    lB_d = nc.declare_dram_parameter("lB", [D + 2, B], f16, isOutput=False)
    xmA_d = nc.declare_dram_parameter("xmA", [128, pcp], f16, isOutput=False)
    xmB_d = nc.declare_dram_parameter("xmB", [D + 2, pcp], f16, isOutput=False)
    out_d = nc.declare_dram_parameter("out", [Q, NQB * C * 8], f32, isOutput=True)

    tiles = _class_tiles(W)
    starts = np.concatenate([[0], np.cumsum(W)]).astype(int)

    with TileContext(nc) as tc:
        with (
            tc.tile_pool(name="const", bufs=1) as const_pool,
            tc.tile_pool(name="psum", bufs=4, space="PSUM") as psum_pool,
            tc.tile_pool(name="small", bufs=3) as small_pool,
        ):
            lA_sb = const_pool.tile([128, B], f16)
            lB_sb = const_pool.tile([D + 2, B], f16)
            xmA_sb = const_pool.tile([128, pcp], f16)
            xmB_sb = const_pool.tile([D + 2, pcp], f16)
            # DMA order: exactly what the first (qblock 0, tile 0) matmuls
            # need, then everything else; xm split on tile boundaries
            h1 = int(starts[2])
            nc.sync.dma_start(out=lA_sb[:, 0:Q], in_=lA_d[:, 0:Q])
            nc.sync.dma_start(out=lB_sb[:, 0:Q], in_=lB_d[:, 0:Q])
            nc.sync.dma_start(out=xmA_sb[:, 0:h1], in_=xmA_d[:, 0:h1])
            nc.sync.dma_start(out=xmB_sb[:, 0:h1], in_=xmB_d[:, 0:h1])
            h2 = int(starts[4])
            nc.sync.dma_start(out=xmA_sb[:, h1:h2], in_=xmA_d[:, h1:h2])
            nc.sync.dma_start(out=xmB_sb[:, h1:h2], in_=xmB_d[:, h1:h2])
            nc.sync.dma_start(out=lA_sb[:, Q:B], in_=lA_d[:, Q:B])
            nc.sync.dma_start(out=lB_sb[:, Q:B], in_=lB_d[:, Q:B])
            nc.sync.dma_start(out=xmA_sb[:, h2:pcp], in_=xmA_d[:, h2:pcp])
            nc.sync.dma_start(out=xmB_sb[:, h2:pcp], in_=xmB_d[:, h2:pcp])

            for b in range(NQB):
                lAb = lA_sb[:, b * Q : (b + 1) * Q]
                lBb = lB_sb[:, b * Q : (b + 1) * Q]
                candb = small_pool.tile([Q, C * 8], f32, tag="cand")
                for (cls, tw) in tiles:
                    t0 = int(starts[cls[0]])
                    ps = psum_pool.tile([Q, tw], f32, tag="ps")
                    pos = 0
                    while pos < tw:
                        w = min(CHUNK, tw - pos)
                        sl = slice(pos, pos + w)
                        rsl = slice(t0 + pos, t0 + pos + w)
                        nc.tensor.matmul(
                            ps[:, sl], lhsT=lAb, rhs=xmA_sb[:, rsl],
                            start=True, stop=False,
                        )
                        nc.tensor.matmul(
                            ps[:, sl], lhsT=lBb, rhs=xmB_sb[:, rsl],
                            start=False, stop=True,
                        )
                        pos += w
                    if b == 0 and cls[0] == 0:
                        src = ps   # first tile: straight off PSUM (no Act
                                   # stage on the pipeline-fill critical path)
                    else:
                        src = small_pool.tile([Q, tw], f32, tag="ksb")
                        nc.scalar.copy(src, ps)
                    for c in cls:
                        rs = int(starts[c]) - t0
                        nc.vector.max(
                            out=candb[:, c * 8 : (c + 1) * 8],
                            in_=src[:, rs : rs + W[c]],
                        )
                bs = b * C * 8
                nc.sync.dma_start(out=out_d[:, bs : bs + C * 8], in_=candb)

    nc.finalize()
    return nc


def _prepare(x, X_train, y_train):
    perm, counts, q_c, W, starts, pc, pcp = _plan_layout(y_train)
    Xs = X_train[perm].astype(np.float32)
    t_sq = np.sum(Xs * Xs, axis=1)

    # per-core augmented columns [core][65, pcp]
    that = np.zeros((NCORES, D + 1, pcp), dtype=np.float32)
    that[:, D, :] = NEG_PAD
    cstart = np.concatenate([[0], np.cumsum(counts)]).astype(int)
    for c in range(C):
        for k in range(NCORES):
            lo = int(cstart[c]) + k * q_c[c]
            hi = min(int(cstart[c]) + (k + 1) * q_c[c], int(cstart[c + 1]))
            n = hi - lo
            if n <= 0:
                continue
            s = int(starts[c])
            that[k, :D, s : s + n] = Xs[lo:hi].T
            that[k, D, s : s + n] = -0.5 * t_sq[lo:hi]

    th = that.astype(np.float16)
    tl = (that - th.astype(np.float32)).astype(np.float16)
    xmA = np.concatenate([th, th[:, 0:63]], axis=1)      # [8, 128, pcp]
    xmB = np.concatenate([tl, th[:, 63:64]], axis=1)     # [8, 66, pcp]

    xhat = np.concatenate([x.T.astype(np.float32), np.ones((1, B), np.float32)], axis=0)
    xh = xhat.astype(np.float16)
    xl = (xhat - xh.astype(np.float32)).astype(np.float16)
    lA = np.concatenate([xh, xl[0:63]], axis=0)          # [128, B]
    lB = np.concatenate([xh, xl[63:64]], axis=0)         # [66, B]
    return xmA, xmB, lA, lB, W, pcp


def kernel(x: np.ndarray, X_train: np.ndarray, y_train: np.ndarray) -> np.ndarray:
    global _compiled, _compiled_key
    from concourse.bass_utils import run_bass_kernel_spmd

    x = np.asarray(x, dtype=np.float32)
    X_train = np.asarray(X_train, dtype=np.float32)
    y_train = np.asarray(y_train)

    xmA, xmB, lA, lB, W, pcp = _prepare(x, X_train, y_train)

    key = (pcp, tuple(W))
    if _compiled is None or _compiled_key != key:
        _compiled = _build_nc(W, pcp)
        _compiled_key = key
    nc = _compiled

    in_maps = [
        {"lA": lA, "lB": lB, "xmA": xmA[k], "xmB": xmB[k]}
        for k in range(NCORES)
    ]
    res = run_bass_kernel_spmd(nc, in_maps, core_ids=list(range(NCORES)))

    # host merge: [cores][Q, NQB*C*8] -> per query 8*C*8 candidates
    outs = np.stack([res.results[k]["out"] for k in range(NCORES)])  # [8,Q,NQB*C*8]
    outs = outs.reshape(NCORES, Q, NQB, C * 8).transpose(2, 1, 0, 3)  # [NQB,Q,8,C*8]
    vals = outs.reshape(B, NCORES * C * 8)
    # class of each slot: slots are [core][class][8]
    cls = np.tile(np.repeat(np.arange(C), 8), NCORES)                 # [NCORES*C*8]
    tq = -np.partition(-vals, K - 1, axis=1)[:, K - 1 : K]            # [B,1]
    ge = vals >= tq                                                   # [B, S]
    counts = np.stack([np.sum(ge[:, cls == c], axis=1) for c in range(C)], axis=1)
    counts = counts.astype(np.float32)
    out = counts / np.sum(counts, axis=-1, keepdims=True)
    return out.astype(np.float32)
